# revision 14
# baseline (speedup 1.0000x reference)
import sys
import numpy as np

try:
    import concourse
except ImportError:
    for _p in ("/opt/trn_rl_repo", "/root/.axon_site/_ro/trn_rl_repo"):
        if _p not in sys.path:
            sys.path.append(_p)

N_ROWS = 16384
D = 128
N_CORES = 8
R = N_ROWS // N_CORES
P = 128
Q = R // P
C_CONST = 1.0 / (N_ROWS - 1)
RCP_MAGIC = 0x7EF311C3
SQ0 = Q
NS = 2 * Q + 1

W = "wait"
G = "sig"
CFG = {
    "sched": (
        ("sp", "dma", 0, 6, {G: "c0"}),
        ("sp", "dma", 6, 12, {G: "c1"}),
        ("sp", "dma", 12, 17, {G: "c2"}),
        ("sp", "clr", {}),
        ("act", "dummy", {}),
        ("dve", "pr", 0, 5, {W: ("c0",)}),
        ("act", "sq", 0, 6, {W: ("c0",), G: "sqA"}),
        ("dve", "f1", 0, 5, {}),
        ("dve", "pr", 5, 11, {W: ("c1",)}),
        ("act", "sq", 6, 9, {W: ("c1",), G: "sqB"}),
        ("pool", "sq", 9, 12, {W: ("c1",), G: "sqC"}),
        ("pool", "f1", 16, 22, {W: ("sqA",)}),
        ("pool", "f1", 25, 28, {W: ("sqC",), G: "fA"}),
        ("dve", "f1", 5, 11, {}),
        ("dve", "pr", 11, 16, {W: ("c2",)}),
        ("act", "sq", 12, 17, {W: ("c2",), G: "sqD"}),
        ("dve", "f1", 11, 16, {}),
        ("dve", "f2", 0, 16, {}),
        ("dve", "f3", 0, 16, {}),
        ("dve", "red", 0, 16, 3, {}),
        ("dve", "f1", 22, 25, {W: ("sqB",)}),
        ("pool", "f1", 28, 33, {W: ("sqD",), G: "fC"}),
        ("dve", "f2", 16, 28, {W: ("fA",)}),
        ("dve", "f3", 16, 28, {}),
        ("dve", "red", 16, 28, 3, {}),
        ("dve", "f2", 28, 33, {W: ("fC",)}),
        ("dve", "red", 28, 33, 2, {G: "rdone"}),
        ("dve", "fin", 0, 16, {W: ("rdone",), G: "fin_done"}),
        ("sp", "store", {W: ("fin_done",), G: "stored"}),
        ("sp", "end", {W: ("stored",)}),
    ),
    "strip_barrier": True,
}

_CACHE = {}


def _build_nc(cfg=None):
    import concourse.bacc as bacc
    import bass_rust as _br
    from concourse import mybir

    cfg = dict(CFG, **(cfg or {}))
    f32 = mybir.dt.float32
    bf16 = mybir.dt.bfloat16
    i32 = mybir.dt.int32
    AF = mybir.ActivationFunctionType
    ALU = mybir.AluOpType

    sched = cfg["sched"]

    prod, sqs, acc, red_cov, fin_cov = set(), set(), set(), set(), set()
    red_lvl = {}
    lvl_cov = {1: set(), 2: set(), 3: set()}
    for entry in sched:
        _eng, kind, *rest = entry
        args = rest[:-1]
        if kind == "pr":
            prod |= set(range(args[0], args[1]))
        elif kind == "sq":
            sqs |= set(range(SQ0 + args[0], SQ0 + args[1]))
        elif kind == "sqa":
            acc.add(SQ0 + args[0])
        elif kind in ("f1", "f2", "f3"):
            lvl_cov[int(kind[1])] |= set(range(args[0], args[1]))
        elif kind == "red":
            red_cov |= set(range(args[0], args[1]))
            for s in range(args[0], args[1]):
                red_lvl[s] = args[2]
        elif kind == "fin":
            fin_cov |= set(range(args[0], args[1]))
        elif kind == "clr":
            pass
    lad = prod | sqs
    if not cfg.get("skip_cov_check"):
        assert prod == set(range(Q))
        assert sqs | {s for s in acc if s >= Q} == set(range(SQ0, NS))
        assert red_cov == lad
        for s in lad:
            for lvl in range(1, red_lvl[s] + 1):
                assert s in lvl_cov[lvl], (s, lvl)
        assert fin_cov == set(range(Q))

    nc = bacc.Bacc(
        "TRN2",
        target_bir_lowering=False,
        debug=False,
        enable_asserts=False,
        num_devices=N_CORES,
    )
    xc = nc.dram_tensor("xc", [P, (Q + 1) * D], bf16, kind="ExternalInput").ap()
    out_pq = nc.dram_tensor("out", [P, Q], f32, kind="ExternalOutput").ap()

    AB = nc.alloc_sbuf_tensor("AB", [P, (Q + 1) * D], bf16).ap()
    PRSQ = nc.alloc_sbuf_tensor("PRSQ", [P, NS * D], bf16).ap()
    F = [None,
         nc.alloc_sbuf_tensor("F1", [P, NS * 64], bf16).ap(),
         nc.alloc_sbuf_tensor("F2", [P, NS * 32], bf16).ap(),
         nc.alloc_sbuf_tensor("F3", [P, NS * 16], bf16).ap()]
    SEG = nc.alloc_sbuf_tensor("SEG", [P, NS], f32).ap()
    S = SEG[:, 0:Q]
    NE = SEG[:, SQ0:NS]
    OUT = nc.alloc_sbuf_tensor("OUT", [P, Q], f32).ap()
    MG = nc.alloc_sbuf_tensor("MG", [P, Q], f32).ap()
    WW = nc.alloc_sbuf_tensor("WW", [P, Q], f32).ap()
    UU = nc.alloc_sbuf_tensor("UU", [P, Q], f32).ap()
    VV = nc.alloc_sbuf_tensor("VV", [P, Q], f32).ap()
    AD = nc.alloc_sbuf_tensor("AD", [P, Q], f32).ap()
    R0 = nc.alloc_sbuf_tensor("R0", [P, Q], f32).ap()
    T1 = nc.alloc_sbuf_tensor("T1", [P, Q], f32).ap()
    H1 = nc.alloc_sbuf_tensor("H1", [P, Q], f32).ap()
    U2 = nc.alloc_sbuf_tensor("U2", [P, Q], f32).ap()
    CR = nc.alloc_sbuf_tensor("CR", [P, Q], i32).ap()
    TWO = nc.alloc_sbuf_tensor("TWO", [P, Q], f32).ap()
    CB = nc.alloc_sbuf_tensor("CB", [P, Q], f32).ap()
    PO = nc.alloc_sbuf_tensor("PO", [P, 2 * D], bf16).ap()
    DUM = nc.alloc_sbuf_tensor("DUM", [P, 1], f32).ap()

    engs = {"dve": nc.vector, "pool": nc.gpsimd, "act": nc.scalar,
            "sp": nc.sync}
    nc.vector.memset(CR, RCP_MAGIC)
    nc.vector.memset(TWO, 2.0)
    nc.vector.memset(CB, C_CONST * D)

    sems = {}
    sem_val = {}

    def get_sem(name):
        if name not in sems:
            sems[name] = nc.alloc_semaphore(name)
        return sems[name]

    for entry in sched:
        meta0 = entry[-1]
        for nm in meta0.get(W, ()):
            get_sem(nm)
        if G in meta0:
            get_sem(meta0[G])
    sem_nums = [s.num for s in sems.values()]
    sem_range = range(min(sem_nums), max(sem_nums) + 1)
    for _pad in range(7):
        nc.vector.memset(PO[:, _pad * 16:(_pad + 1) * 16], 0.0)
    for _pad in range(3):
        nc.gpsimd.memset(PO[:, D + _pad * 16:D + (_pad + 1) * 16], 0.0)

    def emit_fin(ga, gb):
        nea = NE[:, ga:gb]
        neb = NE[:, ga + 1:gb + 1]
        nc.vector.tensor_tensor(out=MG[:, ga:gb], in0=nea, in1=neb,
                                op=ALU.mult)
        nc.vector.tensor_tensor(out=UU[:, ga:gb], in0=S[:, ga:gb],
                                in1=CB[:, ga:gb], op=ALU.add)
        nc.vector.tensor_tensor(out=R0[:, ga:gb].bitcast(i32),
                                in0=CR[:, ga:gb],
                                in1=MG[:, ga:gb].bitcast(i32),
                                op=ALU.subtract)
        nc.vector.tensor_tensor(out=T1[:, ga:gb], in0=MG[:, ga:gb],
                                in1=R0[:, ga:gb], op=ALU.mult)
        nc.vector.tensor_tensor(out=VV[:, ga:gb], in0=UU[:, ga:gb],
                                in1=UU[:, ga:gb], op=ALU.mult)
        nc.vector.tensor_tensor(out=H1[:, ga:gb], in0=TWO[:, ga:gb],
                                in1=T1[:, ga:gb], op=ALU.subtract)
        nc.vector.tensor_tensor(out=WW[:, ga:gb], in0=R0[:, ga:gb],
                                in1=H1[:, ga:gb], op=ALU.mult)
        return nc.vector.tensor_tensor(out=OUT[:, ga:gb], in0=VV[:, ga:gb],
                                       in1=WW[:, ga:gb], op=ALU.mult)

    for entry in sched:
        eng_name, kind, *rest = entry
        meta = rest[-1]
        args = rest[:-1]
        e = engs[eng_name]
        for wname in meta.get(W, ()):
            e.wait_ge(get_sem(wname), sem_val[wname])
        inst = None
        if kind == "clr":
            nc.sync.sem_clear(sem_range)
        elif kind == "dma":
            lo, hi = args
            inst = nc.sync.dma_start(out=AB[:, lo * D:hi * D],
                                     in_=xc[:, lo * D:hi * D])
        elif kind == "dummy":
            inst = nc.scalar.activation(out=DUM, in_=DUM, func=AF.Square)
        elif kind == "pr":
            a, b = args
            inst = e.tensor_tensor(
                out=PRSQ[:, a * D:b * D], in0=AB[:, a * D:b * D],
                in1=AB[:, a * D + D:b * D + D], op=ALU.mult)
        elif kind == "sq":
            a, b = args
            dst = PRSQ[:, (SQ0 + a) * D:(SQ0 + b) * D]
            src = AB[:, a * D:b * D]
            if eng_name == "act":
                inst = nc.scalar.activation(out=dst, in_=src, func=AF.Square)
            else:
                inst = e.tensor_tensor(out=dst, in0=src, in1=src, op=ALU.mult)
        elif kind == "sqa":
            t, = args
            inst = nc.scalar.activation(
                out=PO[:, D:2 * D], in_=AB[:, t * D:(t + 1) * D],
                func=AF.Square, accum_out=SEG[:, SQ0 + t:SQ0 + t + 1])
        elif kind in ("f1", "f2", "f3"):
            lvl = int(kind[1])
            a, b = args
            w = 128 >> lvl
            src = PRSQ if lvl == 1 else F[lvl - 1]
            nseg = b - a
            c3 = src[:, a * 2 * w:b * 2 * w].rearrange(
                "p (q d) -> p q d", q=nseg)
            n3 = F[lvl][:, a * w:b * w].rearrange("p (q d) -> p q d", q=nseg)
            inst = e.tensor_tensor(out=n3, in0=c3[:, :, 0:w],
                                   in1=c3[:, :, w:2 * w], op=ALU.add)
        elif kind == "red":
            a, b, lvl = args
            w = 128 >> lvl
            nseg = b - a
            inst = nc.vector.tensor_reduce(
                SEG[:, a:b],
                F[lvl][:, a * w:b * w].rearrange("p (q d) -> p q d", q=nseg),
                axis=mybir.AxisListType.X, op=ALU.add)
        elif kind == "fin":
            ga, gb = args
            inst = emit_fin(ga, gb)
        elif kind == "store":
            inst = nc.sync.dma_start(out=out_pq, in_=OUT)
        elif kind == "end":
            nc.sync.sem_clear(sem_range)
            inst = None
        else:
            raise AssertionError(entry)
        if G in meta:
            val = 16 if kind in ("dma", "store") else 1
            inst.then_inc(get_sem(meta[G]), val)
            sem_val[meta[G]] = val

    blk0 = nc.m.functions[0].blocks[0]
    last_by_engine = {}
    for inst in blk0.instructions:
        eng = str(inst.engine)
        if eng in last_by_engine:
            dep = _br.InstructionNameOrderedSet()
            dep.add(last_by_engine[eng])
            inst.add_nosync_dependencies_from(dep)
        last_by_engine[eng] = inst.name

    if cfg.get("strip_barrier", True):
        from concourse import mybir as _mb
        for inst in blk0.instructions:
            if inst.opcode in ("Drain", "EventSemaphore") and (
                    inst.name.startswith("barrier_") or inst.name in
                    ("I-38", "I-40", "I-42", "I-44", "I-46")):
                inst.sync_info = _mb.SyncInfo(on_wait=[], on_update=[])

    nc.compile()
    return nc


def _get_nc():
    if "nc" not in _CACHE:
        _CACHE["nc"] = _build_nc()
    return _CACHE["nc"]


def make_in_maps(x: np.ndarray) -> list[dict[str, np.ndarray]]:
    import ml_dtypes

    x = np.asarray(x, dtype=np.float32)
    xp = np.concatenate([x, np.ones((1, D), dtype=np.float32)], axis=0)
    xp = xp.astype(ml_dtypes.bfloat16)
    in_maps = []
    for c in range(N_CORES):
        sh = xp[c * R:c * R + R].reshape(P, Q * D)
        halo = xp[c * R + 16 * np.arange(1, P + 1)]
        xcm = np.concatenate([sh, halo.reshape(P, D)], axis=1)
        in_maps.append({"xc": np.ascontiguousarray(xcm)})
    return in_maps


def kernel(feature_clusters: np.ndarray) -> np.ndarray:
    from concourse.bass_utils import run_bass_kernel_spmd

    nc = _get_nc()
    in_maps = make_in_maps(feature_clusters)

    def run_once():
        res = run_bass_kernel_spmd(nc, in_maps, list(range(N_CORES))).results
        return np.concatenate(
            [res[c]["out"].reshape(R) for c in range(N_CORES)])

    prev = run_once()
    for _ in range(5):
        cur = run_once()
        if np.array_equal(prev, cur) and np.isfinite(cur).all():
            break
        prev = cur
    return prev[:N_ROWS - 1].astype(np.float32)


# revision 16
# speedup vs baseline: 1.0097x; 1.0097x over previous
import sys
import numpy as np

try:
    import concourse
except ImportError:
    for _p in ("/opt/trn_rl_repo", "/root/.axon_site/_ro/trn_rl_repo"):
        if _p not in sys.path:
            sys.path.append(_p)

N_ROWS = 16384
D = 128
N_CORES = 8
R = N_ROWS // N_CORES
P = 128
Q = R // P
C_CONST = 1.0 / (N_ROWS - 1)
RCP_MAGIC = 0x7EF311C3
SQ0 = Q
NS = 2 * Q + 1

W = "wait"
G = "sig"
CFG = {
    "sched": (
        ("sp", "dma", 0, 6, {G: "c0"}),
        ("sp", "dma", 6, 12, {G: "c1"}),
        ("sp", "dma", 12, 17, {G: "c2"}),
        ("sp", "clr", {}),
        ("act", "dummy", {}),
        ("dve", "pr", 0, 5, {W: ("c0",)}),
        ("act", "sq", 0, 6, {W: ("c0",), G: "sqA"}),
        ("dve", "f1", 0, 5, {}),
        ("dve", "pr", 5, 11, {W: ("c1",)}),
        ("act", "sq", 6, 9, {W: ("c1",), G: "sqB"}),
        ("pool", "sq", 9, 12, {W: ("c1",), G: "sqC"}),
        ("pool", "f1", 16, 22, {W: ("sqA",)}),
        ("pool", "f1", 25, 28, {W: ("sqC",), G: "fA"}),
        ("dve", "f1", 5, 11, {}),
        ("dve", "pr", 11, 16, {W: ("c2",)}),
        ("act", "sq", 12, 17, {W: ("c2",), G: "sqD"}),
        ("dve", "f1", 11, 16, {}),
        ("dve", "f2", 0, 16, {}),
        ("dve", "f3", 0, 16, {}),
        ("dve", "red", 0, 16, 3, {}),
        ("dve", "f1", 22, 25, {W: ("sqB",)}),
        ("pool", "f1", 28, 33, {W: ("sqD",), G: "fC"}),
        ("dve", "f2", 16, 28, {W: ("fA",)}),
        ("dve", "f3", 16, 28, {}),
        ("dve", "red", 16, 28, 3, {}),
        ("dve", "f2", 28, 33, {W: ("fC",)}),
        ("dve", "red", 28, 33, 2, {G: "rdone"}),
        ("dve", "fine", 0, 16, {}),
        ("dve", "fin", 0, 16, {W: ("rdone",), G: "fin_done"}),
        ("sp", "store", {W: ("fin_done",), G: "stored"}),
        ("sp", "end", {W: ("stored",)}),
    ),
    "strip_barrier": True,
}

_CACHE = {}


def _build_nc(cfg=None):
    import concourse.bacc as bacc
    import bass_rust as _br
    from concourse import mybir

    cfg = dict(CFG, **(cfg or {}))
    f32 = mybir.dt.float32
    bf16 = mybir.dt.bfloat16
    i32 = mybir.dt.int32
    AF = mybir.ActivationFunctionType
    ALU = mybir.AluOpType

    sched = cfg["sched"]

    prod, sqs, acc, red_cov, fin_cov = set(), set(), set(), set(), set()
    red_lvl = {}
    lvl_cov = {1: set(), 2: set(), 3: set()}
    for entry in sched:
        _eng, kind, *rest = entry
        args = rest[:-1]
        if kind == "pr":
            prod |= set(range(args[0], args[1]))
        elif kind == "sq":
            sqs |= set(range(SQ0 + args[0], SQ0 + args[1]))
        elif kind == "sqa":
            acc.add(SQ0 + args[0])
        elif kind in ("f1", "f2", "f3"):
            lvl_cov[int(kind[1])] |= set(range(args[0], args[1]))
        elif kind == "red":
            red_cov |= set(range(args[0], args[1]))
            for s in range(args[0], args[1]):
                red_lvl[s] = args[2]
        elif kind == "fin":
            fin_cov |= set(range(args[0], args[1]))
        elif kind in ("clr", "fine"):
            pass
    lad = prod | sqs
    if not cfg.get("skip_cov_check"):
        assert prod == set(range(Q))
        assert sqs | {s for s in acc if s >= Q} == set(range(SQ0, NS))
        assert red_cov == lad
        for s in lad:
            for lvl in range(1, red_lvl[s] + 1):
                assert s in lvl_cov[lvl], (s, lvl)
        assert fin_cov == set(range(Q))

    nc = bacc.Bacc(
        "TRN2",
        target_bir_lowering=False,
        debug=False,
        enable_asserts=False,
        num_devices=N_CORES,
    )
    xc = nc.dram_tensor("xc", [P, (Q + 1) * D], bf16, kind="ExternalInput").ap()
    out_pq = nc.dram_tensor("out", [P, Q], f32, kind="ExternalOutput").ap()

    AB = nc.alloc_sbuf_tensor("AB", [P, (Q + 1) * D], bf16).ap()
    PRSQ = nc.alloc_sbuf_tensor("PRSQ", [P, NS * D], bf16).ap()
    F = [None,
         nc.alloc_sbuf_tensor("F1", [P, NS * 64], bf16).ap(),
         nc.alloc_sbuf_tensor("F2", [P, NS * 32], bf16).ap(),
         nc.alloc_sbuf_tensor("F3", [P, NS * 16], bf16).ap()]
    SEG = nc.alloc_sbuf_tensor("SEG", [P, NS], f32).ap()
    S = SEG[:, 0:Q]
    NE = SEG[:, SQ0:NS]
    OUT = nc.alloc_sbuf_tensor("OUT", [P, Q], f32).ap()
    MG = nc.alloc_sbuf_tensor("MG", [P, Q], f32).ap()
    WW = nc.alloc_sbuf_tensor("WW", [P, Q], f32).ap()
    UU = nc.alloc_sbuf_tensor("UU", [P, Q], f32).ap()
    VV = nc.alloc_sbuf_tensor("VV", [P, Q], f32).ap()
    AD = nc.alloc_sbuf_tensor("AD", [P, Q], f32).ap()
    R0 = nc.alloc_sbuf_tensor("R0", [P, Q], f32).ap()
    T1 = nc.alloc_sbuf_tensor("T1", [P, Q], f32).ap()
    H1 = nc.alloc_sbuf_tensor("H1", [P, Q], f32).ap()
    U2 = nc.alloc_sbuf_tensor("U2", [P, Q], f32).ap()
    CR = nc.alloc_sbuf_tensor("CR", [P, Q], i32).ap()
    TWO = nc.alloc_sbuf_tensor("TWO", [P, Q], f32).ap()
    CB = nc.alloc_sbuf_tensor("CB", [P, Q], f32).ap()
    PO = nc.alloc_sbuf_tensor("PO", [P, 2 * D], bf16).ap()
    DUM = nc.alloc_sbuf_tensor("DUM", [P, 1], f32).ap()

    engs = {"dve": nc.vector, "pool": nc.gpsimd, "act": nc.scalar,
            "sp": nc.sync}
    nc.vector.memset(CR, RCP_MAGIC)
    nc.vector.memset(TWO, 2.0)
    nc.vector.memset(CB, C_CONST * D)

    sems = {}
    sem_val = {}

    def get_sem(name):
        if name not in sems:
            sems[name] = nc.alloc_semaphore(name)
        return sems[name]

    for entry in sched:
        meta0 = entry[-1]
        for nm in meta0.get(W, ()):
            get_sem(nm)
        if G in meta0:
            get_sem(meta0[G])
    sem_nums = [s.num for s in sems.values()]
    sem_range = range(min(sem_nums), max(sem_nums) + 1)
    for _pad in range(7):
        nc.vector.memset(PO[:, _pad * 16:(_pad + 1) * 16], 0.0)
    for _pad in range(3):
        nc.gpsimd.memset(PO[:, D + _pad * 16:D + (_pad + 1) * 16], 0.0)

    def emit_fin_early(ga, gb):
        nc.vector.tensor_tensor(out=UU[:, ga:gb], in0=S[:, ga:gb],
                                in1=CB[:, ga:gb], op=ALU.add)
        return nc.vector.tensor_tensor(out=VV[:, ga:gb], in0=UU[:, ga:gb],
                                       in1=UU[:, ga:gb], op=ALU.mult)

    def emit_fin(ga, gb):
        nea = NE[:, ga:gb]
        neb = NE[:, ga + 1:gb + 1]
        nc.vector.tensor_tensor(out=MG[:, ga:gb], in0=nea, in1=neb,
                                op=ALU.mult)
        nc.vector.tensor_tensor(out=R0[:, ga:gb].bitcast(i32),
                                in0=CR[:, ga:gb],
                                in1=MG[:, ga:gb].bitcast(i32),
                                op=ALU.subtract)
        nc.vector.tensor_tensor(out=T1[:, ga:gb], in0=MG[:, ga:gb],
                                in1=R0[:, ga:gb], op=ALU.mult)
        nc.vector.tensor_tensor(out=H1[:, ga:gb], in0=TWO[:, ga:gb],
                                in1=T1[:, ga:gb], op=ALU.subtract)
        nc.vector.tensor_tensor(out=WW[:, ga:gb], in0=R0[:, ga:gb],
                                in1=H1[:, ga:gb], op=ALU.mult)
        return nc.vector.tensor_tensor(out=OUT[:, ga:gb], in0=VV[:, ga:gb],
                                       in1=WW[:, ga:gb], op=ALU.mult)

    for entry in sched:
        eng_name, kind, *rest = entry
        meta = rest[-1]
        args = rest[:-1]
        e = engs[eng_name]
        for wname in meta.get(W, ()):
            e.wait_ge(get_sem(wname), sem_val[wname])
        inst = None
        if kind == "clr":
            nc.sync.sem_clear(sem_range)
        elif kind == "dma":
            lo, hi = args
            inst = nc.sync.dma_start(out=AB[:, lo * D:hi * D],
                                     in_=xc[:, lo * D:hi * D])
        elif kind == "dummy":
            inst = nc.scalar.activation(out=DUM, in_=DUM, func=AF.Square)
        elif kind == "pr":
            a, b = args
            inst = e.tensor_tensor(
                out=PRSQ[:, a * D:b * D], in0=AB[:, a * D:b * D],
                in1=AB[:, a * D + D:b * D + D], op=ALU.mult)
        elif kind == "sq":
            a, b = args
            dst = PRSQ[:, (SQ0 + a) * D:(SQ0 + b) * D]
            src = AB[:, a * D:b * D]
            if eng_name == "act":
                inst = nc.scalar.activation(out=dst, in_=src, func=AF.Square)
            else:
                inst = e.tensor_tensor(out=dst, in0=src, in1=src, op=ALU.mult)
        elif kind == "sqa":
            t, = args
            inst = nc.scalar.activation(
                out=PO[:, D:2 * D], in_=AB[:, t * D:(t + 1) * D],
                func=AF.Square, accum_out=SEG[:, SQ0 + t:SQ0 + t + 1])
        elif kind in ("f1", "f2", "f3"):
            lvl = int(kind[1])
            a, b = args
            w = 128 >> lvl
            src = PRSQ if lvl == 1 else F[lvl - 1]
            nseg = b - a
            c3 = src[:, a * 2 * w:b * 2 * w].rearrange(
                "p (q d) -> p q d", q=nseg)
            n3 = F[lvl][:, a * w:b * w].rearrange("p (q d) -> p q d", q=nseg)
            inst = e.tensor_tensor(out=n3, in0=c3[:, :, 0:w],
                                   in1=c3[:, :, w:2 * w], op=ALU.add)
        elif kind == "red":
            a, b, lvl = args
            w = 128 >> lvl
            nseg = b - a
            inst = nc.vector.tensor_reduce(
                SEG[:, a:b],
                F[lvl][:, a * w:b * w].rearrange("p (q d) -> p q d", q=nseg),
                axis=mybir.AxisListType.X, op=ALU.add)
        elif kind == "fine":
            ga, gb = args
            inst = emit_fin_early(ga, gb)
        elif kind == "fin":
            ga, gb = args
            inst = emit_fin(ga, gb)
        elif kind == "store":
            inst = nc.sync.dma_start(out=out_pq, in_=OUT)
        elif kind == "end":
            nc.sync.sem_clear(sem_range)
            inst = None
        else:
            raise AssertionError(entry)
        if G in meta:
            val = 16 if kind in ("dma", "store") else 1
            inst.then_inc(get_sem(meta[G]), val)
            sem_val[meta[G]] = val

    blk0 = nc.m.functions[0].blocks[0]
    last_by_engine = {}
    for inst in blk0.instructions:
        eng = str(inst.engine)
        if eng in last_by_engine:
            dep = _br.InstructionNameOrderedSet()
            dep.add(last_by_engine[eng])
            inst.add_nosync_dependencies_from(dep)
        last_by_engine[eng] = inst.name

    if cfg.get("strip_barrier", True):
        from concourse import mybir as _mb
        for inst in blk0.instructions:
            if inst.opcode in ("Drain", "EventSemaphore") and (
                    inst.name.startswith("barrier_") or inst.name in
                    ("I-38", "I-40", "I-42", "I-44", "I-46")):
                inst.sync_info = _mb.SyncInfo(on_wait=[], on_update=[])

    nc.compile()
    return nc


def _get_nc():
    if "nc" not in _CACHE:
        _CACHE["nc"] = _build_nc()
    return _CACHE["nc"]


def make_in_maps(x: np.ndarray) -> list[dict[str, np.ndarray]]:
    import ml_dtypes

    x = np.asarray(x, dtype=np.float32)
    xp = np.concatenate([x, np.ones((1, D), dtype=np.float32)], axis=0)
    xp = xp.astype(ml_dtypes.bfloat16)
    in_maps = []
    for c in range(N_CORES):
        sh = xp[c * R:c * R + R].reshape(P, Q * D)
        halo = xp[c * R + 16 * np.arange(1, P + 1)]
        xcm = np.concatenate([sh, halo.reshape(P, D)], axis=1)
        in_maps.append({"xc": np.ascontiguousarray(xcm)})
    return in_maps


def kernel(feature_clusters: np.ndarray) -> np.ndarray:
    from concourse.bass_utils import run_bass_kernel_spmd

    nc = _get_nc()
    in_maps = make_in_maps(feature_clusters)

    def run_once():
        res = run_bass_kernel_spmd(nc, in_maps, list(range(N_CORES))).results
        return np.concatenate(
            [res[c]["out"].reshape(R) for c in range(N_CORES)])

    prev = run_once()
    for _ in range(5):
        cur = run_once()
        if np.array_equal(prev, cur) and np.isfinite(cur).all():
            break
        prev = cur
    return prev[:N_ROWS - 1].astype(np.float32)
